# revision 28
# baseline (speedup 1.0000x reference)
"""Trainium2 Bass kernel: 4D convolution (kernel 3^4, stride 1, pad 1) + bias.

  out[b,o,t,d,h,w] = bias[o] +
      sum_{i,at,ad,ah,aw} x[b,i,t+at-1,d+ad-1,h+ah-1,w+aw-1] * W[o,i,at,ad,ah,aw]

Shapes: x [2,16,8,8,32,32], W [32,16,3,3,3,3], bias [32] -> out [2,32,8,8,32,32].

Distribution (8 cores): data-parallel over the 16 (b, t) output slices, 2
adjacent t's per core.  Each core gets 4 overlapping d-slabs of its 4-plane
t-slab (host-prepadded h/w halo) plus replicated banded weights, and produces
out[b, :, t0:t0+2].

Per-core algorithm ("2D (t,d)-banded implicit GEMM"):
  * The K (contraction) partition dim packs BOTH conv dims that are banded:
    p = 32*jt + 8*jd + ji with jt in [0,4) (t_in = t0-1+jt), jd in [0,4)
    (d_in = 2*db-1+jd for d-block db), ji in [0,8) (i = 8*ih+ji).
  * The M dim packs two output spatials + channels: m = 64*mt + 32*md + o
    (t_out = t0+mt, d_out = 2*db+md).  Banded weights
        BW[si][(jt,jd,ji), (mt,md,o)] = W[o, 8*ih+ji, jt-mt, jd-md, ah, aw]
    (zero unless jt-mt and jd-md in {0,1,2}) contract (at, ad, i-half) in a
    single matmul: 72 useful MACs per K-column vs 48 for 1D d-banding, so
    only 18 passes si = (ih, ah, aw) instead of 27: 144 matmuls x 512 cols
    = 73.7k PE cycles/core (vs 110.6k for the 1D scheme).
  * 8 PSUM banks = (d-block db in [0,4), h-half hh), each [128, 512] fp32,
    accumulate in place over the 18 passes.  rhs AP =
    XT[db,ih][:, 16*hh+ah : +16, aw : aw+32]  -> N = 16*32 = 512.
  * float32r: 1 PE cycle/row at N >= 256, fp32 PSUM accumulation.
  * Evict PSUM via ScalarE activation (identity + per-partition bias), DMA
    straight out to HBM in the output layout.

The host-side input transforms (overlapping (t,d)-slab extraction + halo
pad, banded weight layout, bias broadcast) are pure data-layout work done in
numpy inside kernel(); the hardware kernel consumes them as external inputs.
"""

import numpy as np

I_C, O_C = 16, 32
B_FULL, T_FULL, D, H, W = 2, 8, 8, 32, 32
HP, WP = H + 2, W + 2
N_CORES = 8
NSTEP = 18  # (ih, ah, aw) K-passes
NDB = 4  # d-blocks of 2 d_out each

_NC_CACHE: list = []


def emit_conv(tc, y_d, xt_d, bw_d, bb_d):
    """Emit the per-core conv program into TileContext `tc`.

    y_d [2, 32, 8, 32, 32] out; xt_d [4, 2, 128, 34, 34] padded x slabs
    (db, ih, p=(jt,jd,ji), h', w'); bw_d [18, 128, 128] banded weights;
    bb_d [128] broadcast bias.
    """
    import concourse.mybir as mybir

    nc = tc.nc
    f32 = mybir.dt.float32
    bf16 = mybir.dt.bfloat16
    Ident = mybir.ActivationFunctionType.Identity
    # dram tensors are declared uint16; reinterpret as bf16 for the PE
    xt_d = xt_d.bitcast(bf16)
    bw_d = bw_d.bitcast(bf16)
    y_d = y_d.bitcast(bf16)

    with (
        tc.tile_pool(name="xpool", bufs=1) as xpool,
        tc.tile_pool(name="wpool", bufs=1) as wpool,
        tc.tile_pool(name="opool", bufs=4) as opool,
        tc.tile_pool(name="ppool", bufs=1, space="PSUM") as ppool,
    ):
        # ---- PSUM accumulators: 8 banks = (db, hh) ----
        acc = {}
        for db in range(NDB):
            for hh in range(2):
                acc[db, hh] = ppool.tile(
                    [128, 512], f32, name=f"acc{db}{hh}", tag=f"acc{db}{hh}"
                )

        # ---- warmup: keep the PE busy (and un-throttle HAM) during the
        # input-DMA lead-in.  Zero matmuls into bank 0; the first real
        # matmul there uses start=True, which discards these results.
        WZ = wpool.tile([128, 128], bf16, name="WZ")
        nc.vector.memset(WZ[:, :], 0.0)
        for _ in range(16):
            nc.tensor.matmul(
                out=acc[0, 0][:, 0:128],
                lhsT=WZ[:, :],
                rhs=WZ[:, :],
                start=True,
                stop=True,
            )

        # ---- weights + x slabs: geometric chunks so the first bank's
        # tiles land first, then the rest ----
        BW = wpool.tile([128, NSTEP * 128], bf16, name="BW")
        BWv = BW.rearrange("p (k m) -> p k m", k=NSTEP)
        bw_r = bw_d  # already [p, k, m] partition-major in dram

        XT = {}
        for db in range(NDB):
            for ih in range(2):
                t = xpool.tile([128, HP * WP], bf16, name=f"XT{db}{ih}")
                XT[db, ih] = t.rearrange("p (h w) -> p h w", h=HP)
        BB = wpool.tile([128, 1], f32, name="BB")

        # Each SWDGE dma_start stripes over a 4-engine SDMA group and its
        # gpsimd launch costs ~570ns, so the first-bank-gating loads are
        # split into partition-halves issued round-robin across groups, and
        # the two small gating weight tiles ride the two HWDGE rings (sync +
        # scalar), which nothing else uses during the lead-in.
        for p in (0, 64):
            nc.gpsimd.dma_start(
                out=XT[0, 0][p : p + 64, 0:19], in_=xt_d[0, 0, p : p + 64, 0:19]
            )
        for p in (0, 64):
            nc.gpsimd.dma_start(out=BWv[p : p + 64, 0:2], in_=bw_r[p : p + 64, 0:2])
        for p in (0, 64):
            nc.gpsimd.dma_start(
                out=BWv[p : p + 64, 2:10], in_=bw_r[p : p + 64, 2:10]
            )
        # the BW k-tail rides the HWDGE rings (needed only from pass 10 on)
        nc.sync.dma_start(out=BWv[0:64, 10:NSTEP], in_=bw_r[0:64, 10:NSTEP])
        nc.scalar.dma_start(out=BWv[64:128, 10:NSTEP], in_=bw_r[64:128, 10:NSTEP])
        nc.sync.dma_start(out=BB[:, :], in_=bb_d.rearrange("(p u) -> p u", u=1))
        for p in (0, 64):
            nc.gpsimd.dma_start(
                out=XT[0, 1][p : p + 64, 0:19], in_=xt_d[0, 1, p : p + 64, 0:19]
            )
        nc.gpsimd.dma_start(out=XT[0, 0][:, 19:HP], in_=xt_d[0, 0, :, 19:HP])
        nc.gpsimd.dma_start(out=XT[0, 1][:, 19:HP], in_=xt_d[0, 1, :, 19:HP])
        for db in range(1, NDB):
            for ih in range(2):
                nc.gpsimd.dma_start(out=XT[db, ih][:, :], in_=xt_d[db, ih])

        # ---- main accumulation, bank-major: each bank's 18 K-passes run
        # consecutively so its eviction overlaps the remaining MM stream ----
        steps = [(ih, ah, aw) for ih in range(2) for ah in range(3) for aw in range(3)]
        last = len(steps) - 1
        for db in range(NDB):
            # both h-halves of this d-block land in one SBUF tile so the
            # out-DMA gets full-(h w) contiguous dram runs
            ot = opool.tile([128, 2 * 512], bf16, name="ot", tag="ot")
            # the very last bank is split into two h-quarter groups of
            # N=256 so the first ships while the second still computes,
            # halving the exposed post-compute tail
            final = db == NDB - 1
            for hh, hq in [(0, None), (1, 0), (1, 1)] if final else [(0, None), (1, None)]:
                if hq is None:
                    rlo, n0, nn = 16 * hh, 0, 512
                else:
                    rlo, n0, nn = 16 * hh + 8 * hq, 256 * hq, 256
                for si, (ih, ah, aw) in enumerate(steps):
                    rhs = XT[db, ih][:, rlo + ah : rlo + ah + nn // 32, aw : aw + W]
                    nc.tensor.matmul(
                        out=acc[db, hh][:, n0 : n0 + nn],
                        lhsT=BWv[:, si, :],
                        rhs=rhs,
                        start=(si == 0),
                        stop=(si == last),
                    )
                nc.scalar.activation(
                    ot[:, 512 * hh + n0 : 512 * hh + n0 + nn],
                    acc[db, hh][:, n0 : n0 + nn],
                    Ident,
                    bias=BB[:, :],
                    scale=1.0,
                )
                if not final:
                    continue
                # final d-block: ship each group (hh0 full-half, then each
                # h-quarter) the moment its activation lands, rings + spray
                # in parallel, so the exposed tail is one N=256 group
                hlo = rlo
                for mt in range(2):
                    for md in range(2):
                        if mt == 0:
                            eng = nc.sync if md == 0 else nc.scalar
                        else:
                            eng = nc.gpsimd
                        ydst = y_d[
                            mt, :, 2 * db + md, hlo : hlo + nn // 32, :
                        ].rearrange("o h w -> o (h w)")
                        p0 = 64 * mt + 32 * md
                        eng.dma_start(
                            out=ydst,
                            in_=ot[p0 : p0 + 32, 512 * hh + n0 : 512 * hh + n0 + nn],
                        )
            if final:
                continue
            # db0 evicts while the SWDGE engines still carry input loads, so
            # it ships on the (by then idle) HWDGE rings; db1/db2 spray in 4
            # quadrant chunks striping across the SDMA engine-groups
            for mt in range(2):
                for md in range(2):
                    if db == 0:
                        eng = nc.sync if md == 0 else nc.scalar
                    else:
                        eng = nc.gpsimd
                    ydst = y_d[mt, :, 2 * db + md, :, :].rearrange(
                        "o h w -> o (h w)"
                    )
                    p0 = 64 * mt + 32 * md
                    eng.dma_start(out=ydst, in_=ot[p0 : p0 + 32, :])


def build_nc():
    if _NC_CACHE:
        return _NC_CACHE[0]
    import concourse.bacc as bacc
    import concourse.mybir as mybir
    from concourse.tile import TileContext

    f32 = mybir.dt.float32
    u16 = mybir.dt.uint16
    nc = bacc.Bacc("TRN2", target_bir_lowering=False, debug=False, num_devices=N_CORES)
    xt_d = nc.dram_tensor("xt", [NDB, 2, 128, HP, WP], u16, kind="ExternalInput").ap()
    bw_d = nc.dram_tensor("bw", [128, NSTEP, 128], u16, kind="ExternalInput").ap()
    bb_d = nc.dram_tensor("bb", [128], f32, kind="ExternalInput").ap()
    y_d = nc.dram_tensor("y", [2, O_C, D, H, W], u16, kind="ExternalOutput").ap()
    with TileContext(nc) as tc:
        emit_conv(tc, y_d, xt_d, bw_d, bb_d)
    nc.compile()
    _NC_CACHE.append(nc)
    return nc


def build_banded_weights(weight):
    """W [32,16,3,3,3,3] -> bw [18, 128, 128] 2D-banded tiles.

    bw[si=(ih,ah,aw)][32*jt+8*jd+ji, 64*mt+32*md+o] =
        W[o, 8*ih+ji, jt-mt, jd-md, ah, aw]   (when taps valid, else 0).
    """
    bw = np.zeros((NSTEP, 128, 128), dtype=np.float32)
    steps = [(ih, ah, aw) for ih in range(2) for ah in range(3) for aw in range(3)]
    for si, (ih, ah, aw) in enumerate(steps):
        for mt in range(2):
            for md in range(2):
                for at in range(3):
                    for ad in range(3):
                        jt, jd = mt + at, md + ad
                        # [ji, o] block
                        bw[
                            si,
                            32 * jt + 8 * jd : 32 * jt + 8 * jd + 8,
                            64 * mt + 32 * md : 64 * mt + 32 * md + 32,
                        ] = weight[:, 8 * ih : 8 * ih + 8, at, ad, ah, aw].T
    # partition-major [p, k, m] so each partition's load is one
    # contiguous dram run
    return to_bf16(np.ascontiguousarray(bw.transpose(1, 0, 2)))


def to_bf16(a):
    """fp32 -> bf16 bits (round-to-nearest-even), as uint16."""
    v = np.ascontiguousarray(a, dtype=np.float32).view(np.uint32)
    return ((v + 0x7FFF + ((v >> 16) & 1)) >> 16).astype(np.uint16)


def from_bf16(u):
    return (u.astype(np.uint32) << 16).view(np.float32)


def shard_inputs(x, weight, bias):
    """Full inputs -> per-core in_maps (padded (t,d) slabs, banded W, bias)."""
    x = np.ascontiguousarray(np.asarray(x, dtype=np.float32))
    weight = np.ascontiguousarray(np.asarray(weight, dtype=np.float32))
    bias = np.ascontiguousarray(np.asarray(bias, dtype=np.float32))

    bw = build_banded_weights(weight)
    bb = np.ascontiguousarray(np.tile(bias, 4))  # partition m = mt,md,o -> bias[o]

    in_maps = []
    for c in range(N_CORES):
        b = c // 4
        t0 = 2 * (c % 4)
        # padded volume xp[i, jt, dpad, h', w'] with t_in = t0-1+jt,
        # d_in = dpad-1, h = h'-1, w = w'-1; zeros outside the tensor.
        xp = np.zeros((I_C, 4, D + 2, HP, WP), dtype=np.float32)
        lo, hi = t0 - 1, t0 + 3
        slo, shi = max(lo, 0), min(hi, T_FULL)
        xp[:, slo - lo : shi - lo, 1 : 1 + D, 1 : 1 + H, 1 : 1 + W] = x[b, :, slo:shi]
        # xt[db, ih, 32*jt+8*jd+ji, h', w'] = xp[8*ih+ji, jt, 2*db+jd, h', w']
        xt = np.empty((NDB, 2, 128, HP, WP), dtype=np.float32)
        for db in range(NDB):
            for ih in range(2):
                blk = xp[8 * ih : 8 * ih + 8, :, 2 * db : 2 * db + 4]  # [ji,jt,jd,h,w]
                xt[db, ih] = blk.transpose(1, 2, 0, 3, 4).reshape(128, HP, WP)
        in_maps.append({"xt": to_bf16(xt), "bw": bw, "bb": bb})
    return in_maps


def unshard_outputs(results):
    out = np.empty((B_FULL, O_C, T_FULL, D, H, W), dtype=np.float32)
    for c in range(N_CORES):
        b = c // 4
        t0 = 2 * (c % 4)
        y = from_bf16(results[c]["y"])
        out[b, :, t0] = y[0]
        out[b, :, t0 + 1] = y[1]
    return out


def run(inputs, trace=False, **kwargs):
    from concourse.bass_utils import run_bass_kernel_spmd

    nc = build_nc()
    in_maps = shard_inputs(inputs["x"], inputs["weight"], inputs["bias"])
    res = run_bass_kernel_spmd(
        nc, in_maps, core_ids=list(range(N_CORES)), trace=trace, **kwargs
    )
    return unshard_outputs(res.results), res


def kernel(x, weight, bias):
    out, _ = run({"x": x, "weight": weight, "bias": bias})
    return out
